# revision 44
# baseline (speedup 1.0000x reference)
"""Trainium2 Bass kernel for nn_ALayer_v2 (spatial-attention layer).

Math (fc1/fc2 branch is dead code in the reference):
    A1    = relu(conv3x3(x_in, w1, pad=1))          # [B,16,H,W]
    A     = sigmoid(conv3x3(A1, w2, pad=1))         # [B,1,H,W]
    A_box = conv3x3(A, ones, pad=1)                 # [B,1,H,W]
    out   = x_out * A_box                           # broadcast over C

Sharding: pure data parallel, 4 images per core on 8 cores.

Per-core dataflow (bf16 compute for conv stages, f32 final multiply):
  - channel-padded planes [70 rows x 68 cols] in SBUF; interior at rows
    3..66, cols 2..65; ring+guards kept zero so conv taps are uniform
    flat shifts.
  - conv1: 6 matmuls per 512-strip (3 dx-taps x 2 K-chunks) accumulate
    into PSUM [96=(dy at 0/32/64, oc), strip] with dx folded in via
    shifted rhs APs; the three 16-row dy blocks are copied to
    free-dim-stacked planes (u_cat) and reduced on DVE; relu on ACT
    writes the image's interior block of a1.
  - images processed in pairs: conv2 (block-diag lhsT [64, 2], 9 taps
    PSUM-accumulated), sigmoid, separable box filter, then broadcast
    (K=2 one-hot matmul) + f32 multiply + store per pair — so pair 0's
    store/load DMA overlaps pair 1's conv1 compute.
"""

import os

import numpy as np

import concourse.bass as bass
import concourse.tile as tile
from concourse import bacc, mybir
from concourse.bass_utils import run_bass_kernel_spmd

F32 = mybir.dt.float32
BF16 = mybir.dt.bfloat16
AF = mybir.ActivationFunctionType

N_CORES = 8
B, C, H, W = 32, 256, 64, 64
BPC = B // N_CORES  # images per core
KC = 2  # channel chunks of 128
HP, WP = 70, 68  # padded plane alloc (rows x cols)
R0, C0 = 3, 2  # interior start (row, col) inside padded plane
PL0 = 2 * WP  # flat offset of conv-plane start (row 2)
PLN = 66 * WP  # conv-plane flat length (rows 2..67) = 4488

def _make_strips(width):
    strips = []
    p = 0
    while p < PLN:
        l = min(width, PLN - p)
        strips.append((p, l))
        p += l
    return strips


STRIPS = _make_strips(512)       # conv2/sigmoid strips (1 PSUM bank)
STRIPS1K = _make_strips(1024)    # conv1 strips (2 PSUM banks)

LAST_RESULT = {}


def _build_nc():
    nc = bacc.Bacc("TRN2", target_bir_lowering=False, debug=False)

    x_in = nc.declare_dram_parameter("x_in", [BPC, C, H, W], BF16, isOutput=False)
    x_out = nc.declare_dram_parameter("x_out", [BPC, C, H, W], BF16, isOutput=False)
    w1l = nc.declare_dram_parameter("w1l", [128, KC, 3, 96], F32, isOutput=False)
    w2p = nc.declare_dram_parameter("w2p", [64, 9, 2], F32, isOutput=False)
    onesE = nc.declare_dram_parameter("onesE", [2, 2, 128], F32, isOutput=False)
    out_d = nc.declare_dram_parameter("out", [BPC, C, H, W], F32, isOutput=True)

    with tile.TileContext(nc) as tc:
        with (
            tc.tile_pool(name="consts", bufs=1) as consts,
            tc.tile_pool(name="planes", bufs=1) as planes,
            tc.tile_pool(name="xstage", bufs=3) as xstage,
            tc.tile_pool(name="xout", bufs=3) as xoutp,
            tc.tile_pool(name="outp", bufs=2) as outp,
            tc.tile_pool(name="u96p", bufs=2) as u96p,
            tc.tile_pool(name="psu", bufs=2, space="PSUM") as psu,
            tc.tile_pool(name="psa", bufs=2, space="PSUM") as psa,
            tc.tile_pool(name="psb", bufs=1, space="PSUM") as psb,
        ):
            # ---- weights: DMA f32 then cast to bf16 ----
            w1l_f = consts.tile([128, KC, 3, 96], F32, name="w1l_f")
            nc.sync.dma_start(out=w1l_f, in_=w1l[:, :, :, :])
            w1l_b = consts.tile([128, KC, 3, 96], BF16, name="w1l_b")
            nc.scalar.activation(out=w1l_b, in_=w1l_f, func=AF.Copy)

            w2p_f = consts.tile([64, 9, 2], F32, name="w2p_f")
            nc.sync.dma_start(out=w2p_f, in_=w2p[:, :, :])
            w2p_b = consts.tile([64, 9, 2], BF16, name="w2p_b")
            nc.scalar.activation(out=w2p_b, in_=w2p_f, func=AF.Copy)

            onesE_f = consts.tile([2, 2, 128], F32, name="onesE_f")
            nc.sync.dma_start(out=onesE_f, in_=onesE[:, :, :])
            onesE_b = consts.tile([2, 2, 128], BF16, name="onesE_b")
            nc.scalar.activation(out=onesE_b, in_=onesE_f, func=AF.Copy)

            # ---- persistent padded planes: zero only ring/guard/hole
            # regions (interiors are always overwritten before reads) ----
            def zero_frame(t, np_, wr=True):
                # rows 0..2 and 67..69 full width; side cols for rows 3..66
                nc.vector.memset(t[0:np_, 0:3, :], 0.0)
                nc.vector.memset(t[0:np_, 67:70, :], 0.0)
                if wr:
                    nc.vector.memset(t[0:np_, 3:67, 0:2], 0.0)
                    nc.vector.memset(t[0:np_, 3:67, 66:68], 0.0)

            xpads = {}
            for pb in range(2):
                for kc in range(KC):
                    t = planes.tile([128, HP, WP], BF16, name=f"xpad{pb}{kc}")
                    zero_frame(t, 128)
                    xpads[pb, kc] = t
            # dy blocks stacked on the free dim (all partition-base 0):
            # u_cat[:, dy, j, :] = plane row j+2 of dy-block
            u_cat = planes.tile([16, 3, 66, WP], BF16, name="u_cat")
            t1 = planes.tile([16, H, W], BF16, name="t1")
            a1p = {}
            for q in range(2):
                a1p[q] = planes.tile([64, HP, WP], BF16, name=f"a1p{q}")
                nc.gpsimd.memset(a1p[q], 0.0)
            a_pair = planes.tile([2, HP, WP], BF16, name="a_pair")
            zero_frame(a_pair, 2)
            s_sb = planes.tile([2, HP, WP], BF16, name="s_sb")
            zero_frame(s_sb, 2, wr=False)
            abox = planes.tile([2, HP, WP], BF16, name="abox")

            a_flat = a_pair.rearrange("p h w -> p (h w)")
            s_flat = s_sb.rearrange("p h w -> p (h w)")
            bx_flat = abox.rearrange("p h w -> p (h w)")
            u_flat = u_cat.rearrange("p d h w -> p (d h w)")

            for q in range(2):  # image pair q = {2q, 2q+1}
                a1_flat = a1p[q].rearrange("p h w -> p (h w)")
                # ---- conv1 per image in the pair ----
                for j in range(2):
                    b = 2 * q + j
                    pb = b % 2
                    for kc in range(KC):
                        xs = xstage.tile([128, H, W], BF16, name=f"xs{b}{kc}",
                                         tag="xs")
                        nc.sync.dma_start(
                            out=xs, in_=x_in[b, kc * 128:(kc + 1) * 128])
                        nc.vector.tensor_copy(
                            xpads[pb, kc][:, R0:R0 + H, C0:C0 + W], xs)
                    for (p0, L) in STRIPS1K:
                        pu = psu.tile([96, 1024], F32, name=f"pu{b}{p0}",
                                      tag="u")
                        # accumulate in two <=512 halves (one PSUM bank each)
                        for hf in range(2):
                            h0 = 512 * hf
                            hl = min(512, L - h0)
                            if hl <= 0:
                                continue
                            nmm = 0
                            for kc in range(KC):
                                xp_flat = xpads[pb, kc].rearrange(
                                    "p h w -> p (h w)")
                                for dx in range(3):
                                    o = PL0 + p0 + h0 + dx - 1
                                    nc.tensor.matmul(
                                        pu[:, h0:h0 + hl],
                                        w1l_b[:, kc, dx, :],
                                        xp_flat[:, o:o + hl],
                                        start=(nmm == 0), stop=(nmm == 5))
                                    nmm += 1
                        # two-hop dy reshuffle: wide PSUM->SBUF copy (ACT),
                        # then 16-row bf16 SBUF->SBUF copies at DVE 4x mode
                        u96 = u96p.tile([96, 1024], BF16, name=f"u96{b}{p0}",
                                        tag="u96")
                        nc.scalar.activation(out=u96[:, :L], in_=pu[:, :L],
                                             func=AF.Copy)
                        for dy in range(3):
                            nc.vector.tensor_copy(
                                u_flat[:, dy * 66 * WP + p0:
                                       dy * 66 * WP + p0 + L],
                                u96[32 * dy:32 * dy + 16, :L])
                    # dy-reduction on DVE, relu on ACT into a1 interior
                    j0 = R0 - 2
                    nc.vector.tensor_add(
                        t1,
                        u_cat[:, 0, j0 - 1:j0 - 1 + H, C0:C0 + W],
                        u_cat[:, 1, j0:j0 + H, C0:C0 + W])
                    nc.vector.tensor_add(
                        t1, t1, u_cat[:, 2, j0 + 1:j0 + 1 + H, C0:C0 + W])
                    nc.scalar.activation(
                        out=a1p[q][32 * j:32 * j + 16, R0:R0 + H, C0:C0 + W],
                        in_=t1, func=AF.Relu)

                # ---- conv2 (batched over the pair) + sigmoid ----
                for (p0, L) in STRIPS:
                    pa = psa.tile([2, 512], F32, name=f"pa{q}{p0}", tag="a")
                    for t in range(9):
                        dy, dx = t // 3, t % 3
                        off = (dy - 1) * WP + (dx - 1)
                        o = PL0 + p0 + off
                        nc.tensor.matmul(
                            pa[:, :L], w2p_b[:, t, :], a1_flat[:, o:o + L],
                            start=(t == 0), stop=(t == 8))
                    nc.scalar.activation(
                        out=a_flat[0:2, PL0 + p0:PL0 + p0 + L],
                        in_=pa[:, :L], func=AF.Sigmoid)
                # zero the conv ring of A (sigmoid(0)=0.5 leaked into it)
                nc.vector.memset(a_pair[0:2, 2:3, :], 0.0)
                nc.vector.memset(a_pair[0:2, 67:68, :], 0.0)
                nc.vector.memset(a_pair[0:2, 2:68, 0:2], 0.0)
                nc.vector.memset(a_pair[0:2, 2:68, 66:68], 0.0)

                # ---- box filter (separable; dx pass on GPSIMD, dy on DVE) ----
                nc.gpsimd.tensor_add(
                    s_flat[:, PL0:PL0 + PLN],
                    a_flat[:, PL0 - 1:PL0 - 1 + PLN],
                    a_flat[:, PL0 + 1:PL0 + 1 + PLN])
                nc.gpsimd.tensor_add(
                    s_flat[:, PL0:PL0 + PLN],
                    s_flat[:, PL0:PL0 + PLN],
                    a_flat[:, PL0:PL0 + PLN])
                nc.vector.tensor_add(
                    bx_flat[:, PL0:PL0 + PLN],
                    s_flat[:, PL0 - WP:PL0 - WP + PLN],
                    s_flat[:, PL0 + WP:PL0 + WP + PLN])
                nc.vector.tensor_add(
                    bx_flat[:, PL0:PL0 + PLN],
                    bx_flat[:, PL0:PL0 + PLN],
                    s_flat[:, PL0:PL0 + PLN])

                # ---- broadcast + multiply + store (per pair) ----
                for j in range(2):
                    b = 2 * q + j
                    for kc in range(KC):
                        xo = xoutp.tile([128, H, W], BF16, name=f"xo{b}{kc}",
                                        tag="xo")
                        nc.sync.dma_start(
                            out=xo, in_=x_out[b, kc * 128:(kc + 1) * 128])
                        ot = outp.tile([128, H, W], F32, name=f"ot{b}{kc}",
                                       tag="ot")
                        for g in range(4):
                            pbc = psb.tile([128, 16, W], F32,
                                           name=f"bc{b}{kc}{g}", tag="bc")
                            for h2 in range(2):
                                r = R0 + 16 * g + 8 * h2
                                nc.tensor.matmul(
                                    pbc[:, 8 * h2:8 * h2 + 8, :],
                                    onesE_b[0:2, j, :],
                                    abox[0:2, r:r + 8, C0:C0 + W])
                            nc.vector.tensor_mul(
                                ot[:, 16 * g:16 * g + 16, :],
                                xo[:, 16 * g:16 * g + 16, :], pbc)
                        nc.sync.dma_start(
                            out=out_d[b, kc * 128:(kc + 1) * 128], in_=ot)

    nc.compile()
    return nc


def _pack_weights(w1, w2):
    # w1l[c, kc, dx, dy*32+o] = w1[o, kc*128+c, dy, dx] (dy blocks at 0/32/64)
    w1r = np.asarray(w1, dtype=np.float32).reshape(16, KC, 128, 3, 3)
    w1t = np.ascontiguousarray(w1r.transpose(2, 1, 4, 3, 0))  # [c, kc, dx, dy, o]
    w1l = np.zeros((128, KC, 3, 3, 32), dtype=np.float32)
    w1l[:, :, :, :, :16] = w1t
    w1l = w1l.reshape(128, KC, 3, 96)
    # w2p[32*j+i, t, j] = w2[0, i, dy, dx],  t = dy*3+dx  (j = image in pair)
    w2f = np.asarray(w2, dtype=np.float32).reshape(16, 9)
    w2p = np.zeros((64, 9, 2), dtype=np.float32)
    for j in range(2):
        w2p[32 * j:32 * j + 16, :, j] = w2f
    # onesE[k, j, m] = (k == j)
    onesE = np.zeros((2, 2, 128), dtype=np.float32)
    for j in range(2):
        onesE[j, j, :] = 1.0
    return w1l, w2p, onesE


def _install_profile_hook():
    """Provide antenv.axon_hooks (absent in this image) so trace=True works."""
    import sys
    import types
    try:
        from antenv.axon_hooks import get_axon_ntff_profile_hook  # noqa: F401
        return
    except ImportError:
        pass
    import antenv
    from trn_agent_boot.trn_boot import _ntff_profile_via_ctypes
    mod = types.ModuleType("antenv.axon_hooks")
    holder = [_ntff_profile_via_ctypes("/opt/axon/libaxon_pjrt.so")]
    mod.get_axon_ntff_profile_hook = lambda: holder[0]
    mod.set_axon_ntff_profile_hook = lambda h: holder.__setitem__(0, h)
    sys.modules["antenv.axon_hooks"] = mod
    antenv.axon_hooks = mod
    import concourse.bass_utils as bu
    bu.upload_artifacts = lambda tmpdir: f"local:{tmpdir}"


def kernel(x_in, x_out, w1, w2, fc1=None, fc2=None):
    import ml_dtypes
    x_in = np.ascontiguousarray(
        np.asarray(x_in, dtype=np.float32).astype(ml_dtypes.bfloat16))
    x_out = np.ascontiguousarray(
        np.asarray(x_out, dtype=np.float32).astype(ml_dtypes.bfloat16))
    w1l, w2p, onesE = _pack_weights(w1, w2)

    nc = _build_nc()

    in_maps = []
    for i in range(N_CORES):
        sl = slice(i * BPC, (i + 1) * BPC)
        in_maps.append({
            "x_in": x_in[sl],
            "x_out": x_out[sl],
            "w1l": w1l,
            "w2p": w2p,
            "onesE": onesE,
        })

    trace = os.environ.get("KERNEL_TRACE", "0") == "1"
    kwargs = {}
    if trace:
        _install_profile_hook()
        tdir = os.environ.get("KERNEL_TRACE_DIR", "/tmp/ktrace")
        os.makedirs(tdir, exist_ok=True)
        kwargs["tmpdir"] = tdir
    try:
        res = run_bass_kernel_spmd(
            nc, in_maps, core_ids=list(range(N_CORES)), trace=trace, **kwargs,
        )
    except Exception:
        if not trace:
            raise
        import traceback
        traceback.print_exc()
        print("trace run failed; retrying without trace")
        res = run_bass_kernel_spmd(
            nc, in_maps, core_ids=list(range(N_CORES)), trace=False,
        )
    LAST_RESULT["exec_time_ns"] = res.exec_time_ns
    LAST_RESULT["mean_exec_time_ns"] = res.mean_exec_time_ns
    LAST_RESULT["profile_json"] = res.profile_json

    out = np.concatenate([res.results[i]["out"] for i in range(N_CORES)], axis=0)
    return out
